# revision 3
# baseline (speedup 1.0000x reference)
"""Distance-aware comb-pilot interpolator for Trainium2 (8 NeuronCores).

Math: out[b, i, c] = (w_l[i] * H[b, j0(i), c] + w_r[i] * H[b, j1(i), c]) / w[i]
with pilots on the comb loc[k] = 8k (k = 0..511), Nfft = 4096.  For
i = 8k + r the normalized weights depend only on r, so each 128-subcarrier
block of the output is the SAME banded 17x128 matrix W applied to 17
consecutive pilots: out[128m + 8kk + r] = alpha[r] H[16m+kk] + gamma[r]
H[16m+kk+1].  The last block folds the reference's extrapolated virtual
pilot hN = (15/8)H[511] - (7/8)H[510] into per-r coefficients on
H[510]/H[511] (a second 16x128 stationary matrix).

Device kernel (per core, batch-sharded 512 rows): one TensorE matmul per
(channel, m) chunk computes 128 subcarriers x 512 batch into PSUM; DVE and
ACT alternate evacuating psum to fp16 SBUF; chunked HWDGE stores stream the
8.4 MB fp16 output (half the f32 bytes -- the fp16 round-trip costs ~1e-3
relative error against a 2e-2 gate).

v2 schedule notes (from the v1 trace):
- ~17 us of the runtime is fixed wrapper cost (engine-rendezvous barriers,
  per-engine TENSOR_LOADs, and a mandatory ~254-instruction semaphore-reset
  epilogue) -- measured 20.0 us for a trivial load+store kernel.  Only the
  work phase is addressable.
- v1's work phase lost ~11 us: weight loads straggled on the scalar ring
  (first matmul at 12.2 us), stores sat in FIFO behind slow descriptor-
  starved loads (first store byte at 21 us), and SDMA engine 15 (slow when
  SWDGE descriptor rings are in use) dragged the final group 41->47 us.
- v2: ALL weights in one [81, 256] DMA issued first on sync; loads are 6
  sixteen-partition mains (8-16 KB descriptors) + 4 one-partition strips,
  split across the two HWDGE rings (sync/scalar) -- gpsimd/SWDGE is not
  used at all; stores stream in 12 groups (2,2,4 chunks to start the
  stream early, 8-chunk groups at line rate, 4,2,2 so the post-compute
  drain is short).  Store descriptors are 2-8 KB runs; the 16-engine spray
  covers all 128 partitions.
"""

import sys

import numpy as np

for _p in ("/opt/trn_rl_repo", "/root/.axon_site/_ro/trn_rl_repo"):
    if _p not in sys.path:
        sys.path.append(_p)

import concourse.bass as bass
import concourse.tile as tile
from concourse import bacc, mybir
from concourse.bass_utils import run_bass_kernel_spmd

N_CORES = 8
B, NP, NFFT, SPACING = 4096, 512, 4096, 8
B_LOC = B // N_CORES  # batch rows per core
P = 128  # SBUF partitions
NCHUNK = 64  # (ch, m) chunks: ch = q // 32 (re/im), m = q % 32 (128-subcarrier block)
MG = 16  # chunk slots per band

# chunks per output store; small leading groups start the store stream as
# soon as the first psum pair is evacuated, 8-chunk middle groups give
# 8 KB-run descriptors (line rate), small trailing groups keep the
# post-compute drain short.
STORE_GROUPS = [2, 2, 4, 8, 8, 8, 8, 8, 8, 4, 2, 2]
assert sum(STORE_GROUPS) == NCHUNK

_PROGRAM = None


def _band_slot(m: int) -> tuple[int, int]:
    """SBUF band (partition base) and column slot of chunk m.

    Odd chunks (and m=31) live at partitions 0..16, even chunks at
    64..80 -- consecutive m alternate PE row-groups so back-to-back
    matmuls overlap in the array."""
    if m == 31:
        return 0, 15
    return (0, m // 2) if m % 2 else (64, m // 2)


def _build_program():
    """One Bass program, identical on all cores (pure data parallel)."""
    nc = bacc.Bacc("TRN2", target_bir_lowering=False, debug=False)
    f16 = mybir.dt.float16
    f32 = mybir.dt.float32
    # ls[34*ch + 17*band2 + j, m2*512 + b]: pre-gathered chunk layout
    # (band2 0 = odd chunks -> partitions 0..16, band2 1 = even -> 64..80).
    # Each row is one partition's contiguous 16 KB payload.
    ls = nc.dram_tensor("ls", [68, MG * B_LOC], f16, kind="ExternalInput").ap()
    # wm[81, 256]: cols 0:128 = W17 band at rows 0..16 AND 64..80 (host
    # duplicates); cols 128:256 rows 0..15 = W16 last-chunk band.
    wm = nc.dram_tensor("wm", [81, 2 * P], f16, kind="ExternalInput").ap()
    # out[p, q*512 + b]: subcarrier-position p = 8*kk + r of chunk q = ch*32 + m.
    out = nc.dram_tensor("out", [P, NCHUNK * B_LOC], f16, kind="ExternalOutput").ap()

    with tile.TileContext(nc) as tc:
        with (
            tc.tile_pool(name="wpool", bufs=1) as wpool,
            tc.tile_pool(name="lpool", bufs=1) as lpool,
            tc.psum_pool(name="ppool", bufs=4) as ppool,
            tc.tile_pool(name="opool", bufs=6) as opool,
        ):
            # Stationary weights: one DMA, 81 partitions x 512 B.  W17 sits
            # at PE base partitions 0 and 64 (cols 0:128); the hN-folded
            # last-chunk band W16 at base 0, cols 128:256.
            wb = wpool.tile([81, 2 * P], f16, name="wb", tag="wb")
            nc.sync.dma_start(wb[:], wm)

            # Data tiles: one per (ch, band).  16 slots x 512 cols fp16 =
            # 16 KB per partition.  Mains are [16, cols] (one 8/16 KB
            # descriptor per partition); the 17th row (j=16) of each band
            # goes as a [1, 16 KB] strip (flat APs spray all 16 engines).
            # 17-partition DMA destinations would break the descriptor
            # spray (HW-measured 3 engines), hence the 16+1 split.
            lts = {
                (ch, band): lpool.tile(
                    [81 if band else 17, MG * B_LOC], f16,
                    name=f"ls{ch}_{band}", tag=f"ls{ch}_{band}",
                )
                for ch in range(2)
                for band in (64, 0)
            }

            def row0(ch, band):
                return 34 * ch + (17 if band else 0)

            def main(ring, ch, band, s0, s1):
                lt = lts[(ch, band)]
                cols = slice(s0 * B_LOC, s1 * B_LOC)
                ring.dma_start(
                    lt[band : band + 16, cols],
                    ls[row0(ch, band) : row0(ch, band) + 16, cols],
                )

            def strip(ring, ch, band):
                lt = lts[(ch, band)]
                r = row0(ch, band) + 16
                ring.dma_start(lt[band + 16 : band + 17, :], ls[r : r + 1, :])

            # sync ring: weights (above) then all mains, consumption order
            # (ch0 slots 0-7 of both bands gate the first two matmuls);
            # stores queue on sync after these, by which time the load
            # descriptors have drained.  scalar ring: the four strips (its
            # auto-inserted ACT_TABLE_LOAD precedes them, so strips land
            # ~1.3us after the first mains -- still ahead of the PE).
            main(nc.sync, 0, 64, 0, 8)
            strip(nc.scalar, 0, 64)
            main(nc.sync, 0, 0, 0, 8)
            strip(nc.scalar, 0, 0)
            main(nc.sync, 0, 64, 8, 16)
            main(nc.sync, 0, 0, 8, 16)
            main(nc.sync, 1, 64, 0, 16)
            strip(nc.scalar, 1, 64)
            main(nc.sync, 1, 0, 0, 16)
            strip(nc.scalar, 1, 0)

            q = 0
            pair = 0
            for gn in STORE_GROUPS:
                o = opool.tile([P, gn * B_LOC], f16)
                q0 = q
                for j in range(0, gn, 2):
                    # one 2-bank psum tile per chunk pair: the paired evac
                    # (FD 1024) amortizes the per-op fixed cost and halves
                    # evac semaphore traffic.
                    ps = ppool.tile([P, 2 * B_LOC], f32)
                    for h in range(2):
                        ch, m = q // 32, q % 32
                        band, m2 = _band_slot(m)
                        if m == 31:
                            # last chunk: pilots 496..511, hN-folded band
                            lhsT, nrows = wb[0:16, P : 2 * P], 16
                        else:
                            lhsT, nrows = wb[band : band + 17, 0:P], 17
                        lt = lts[(ch, band)]
                        nc.tensor.matmul(
                            ps[:, h * B_LOC : (h + 1) * B_LOC],
                            lhsT,
                            lt[band : band + nrows, m2 * B_LOC : (m2 + 1) * B_LOC],
                            start=True,
                            stop=True,
                        )
                        q += 1
                    # psum -> fp16 SBUF, alternating engines so consecutive
                    # pairs drain in parallel (DVE ~1.2us, ACT ~1.1us).
                    osl = o[:, j * B_LOC : (j + 2) * B_LOC]
                    if pair % 2 == 0:
                        nc.vector.tensor_copy(osl, ps[:])
                    else:
                        nc.scalar.copy(osl, ps[:])
                    pair += 1
                nc.sync.dma_start(out[:, q0 * B_LOC : q * B_LOC], o[:])
    nc.compile()
    return nc


def _w_mats(decay_param) -> np.ndarray:
    """[81, 256] fp16 weight tensor.  Cols 0:128: W17[j, 8kk+r] = alpha[r]
    (j=kk) / gamma[r] (j=kk+1) at rows 0..16 and duplicated at rows 64..80.
    Cols 128:256 rows 0..15: the last-chunk band (kk=15 columns use the
    hN-folded coefficients on pilots 510/511)."""
    x = float(np.asarray(decay_param).reshape(-1)[0])
    d = float(np.logaddexp(0.0, x))  # softplus
    r = np.arange(SPACING, dtype=np.float64)
    eps = 1e-12
    wl = np.exp(-d * r)
    wr = np.exp(-d * (float(SPACING) - r))
    w = wl + wr + eps
    alpha, gamma = wl / w, wr / w
    # last 8 subcarriers: i = 4088 + r, x0 = 4088, x1 = 4095 (gap of 7);
    # y1 = hN = (15/8) H[511] - (7/8) H[510]
    wl2 = np.exp(-d * r)
    wr2 = np.exp(-d * (7.0 - r))
    w2 = wl2 + wr2 + eps
    c511 = (wl2 + 1.875 * wr2) / w2
    c510 = -0.875 * wr2 / w2
    W17 = np.zeros((17, P), np.float64)
    Wlast = np.zeros((16, P), np.float64)
    cols = np.arange(SPACING)
    for kk in range(16):
        W17[kk, 8 * kk + cols] = alpha
        W17[kk + 1, 8 * kk + cols] = gamma
    for kk in range(15):
        Wlast[kk, 8 * kk + cols] = alpha
        Wlast[kk + 1, 8 * kk + cols] = gamma
    Wlast[14, 120:128] = c510
    Wlast[15, 120:128] = c511
    W = np.zeros((81, 2 * P), np.float16)
    W[0:17, 0:P] = W17.astype(np.float16)
    W[64:81, 0:P] = W17.astype(np.float16)
    W[0:16, P : 2 * P] = Wlast.astype(np.float16)
    return W


def _gather_ls4(shard: np.ndarray) -> np.ndarray:
    """[68, 8192] fp16: row 34*ch + 17*band2 + j holds partition (band2 ?
    0 : 64) + j's payload -- chunk slots side by side, one contiguous 16 KB
    DRAM run per partition (big-descriptor loads)."""
    lsT = shard.transpose(2, 1, 0).astype(np.float16).reshape(2 * NP, B_LOC)
    j = np.arange(17)[:, None]  # [17, 1]
    m_odd = np.array([2 * m2 + 1 for m2 in range(15)] + [31])  # band2=0 slots
    m_even = np.arange(0, 32, 2)  # band2=1 slots
    rows_odd = np.minimum(16 * m_odd[None, :] + j, 2 * NP // 2 - 1)  # clip m31 j=16
    rows_even = 16 * m_even[None, :] + j
    out = np.empty((68, MG * B_LOC), np.float16)
    for ch in range(2):
        base = 512 * ch
        out[34 * ch : 34 * ch + 17] = lsT[base + rows_odd].reshape(17, -1)
        out[34 * ch + 17 : 34 * ch + 34] = lsT[base + rows_even].reshape(17, -1)
    return out


def kernel(LS_ri, pilot_pos=None, decay_param=None, Nfft=None, **_unused):
    global _PROGRAM
    LS_ri = np.asarray(LS_ri, dtype=np.float32)
    Wm = _w_mats(decay_param)

    if _PROGRAM is None:
        _PROGRAM = _build_program()
    nc = _PROGRAM

    in_maps = []
    for c in range(N_CORES):
        shard = LS_ri[c * B_LOC : (c + 1) * B_LOC]  # [512, 512, 2]
        in_maps.append({"ls": _gather_ls4(shard), "wm": Wm})

    res = run_bass_kernel_spmd(nc, in_maps, list(range(N_CORES))).results
    outs = []
    for c in range(N_CORES):
        a = np.asarray(res[c]["out"]).reshape(16, 8, 2, 32, B_LOC)  # kk r ch m b
        a = a.transpose(4, 3, 0, 1, 2).reshape(B_LOC, NFFT, 2)
        outs.append(a.astype(np.float32))
    return np.concatenate(outs, axis=0)


# revision 5
# speedup vs baseline: 1.0054x; 1.0054x over previous
"""Distance-aware comb-pilot interpolator for Trainium2 (8 NeuronCores).

Math: out[b, i, c] = (w_l[i] * H[b, j0(i), c] + w_r[i] * H[b, j1(i), c]) / w[i]
with pilots on the comb loc[k] = 8k (k = 0..511), Nfft = 4096.  For
i = 8k + r the normalized weights depend only on r, so each 128-subcarrier
block of the output is the SAME banded 17x128 matrix W applied to 17
consecutive pilots: out[128m + 8kk + r] = alpha[r] H[16m+kk] + gamma[r]
H[16m+kk+1].  The last block folds the reference's extrapolated virtual
pilot hN = (15/8)H[511] - (7/8)H[510] into per-r coefficients on
H[510]/H[511] (a second 16x128 stationary matrix).

Device kernel (per core, batch-sharded 512 rows): one TensorE matmul per
(channel, m) chunk computes 128 subcarriers x 512 batch into PSUM; DVE and
ACT alternate evacuating psum to fp16 SBUF; chunked HWDGE stores stream the
8.4 MB fp16 output (half the f32 bytes -- the fp16 round-trip costs ~1e-3
relative error against a 2e-2 gate).

v2 schedule notes (from the v1 trace):
- ~17 us of the runtime is fixed wrapper cost (engine-rendezvous barriers,
  per-engine TENSOR_LOADs, and a mandatory ~254-instruction semaphore-reset
  epilogue) -- measured 20.0 us for a trivial load+store kernel.  Only the
  work phase is addressable.
- v1's work phase lost ~11 us: weight loads straggled on the scalar ring
  (first matmul at 12.2 us), stores sat in FIFO behind slow descriptor-
  starved loads (first store byte at 21 us), and SDMA engine 15 (slow when
  SWDGE descriptor rings are in use) dragged the final group 41->47 us.
- v2: ALL weights in one [81, 256] DMA issued first on sync; loads are 6
  sixteen-partition mains (8-16 KB descriptors) + 4 one-partition strips,
  split across the two HWDGE rings (sync/scalar) -- gpsimd/SWDGE is not
  used at all; stores stream in 12 groups (2,2,4 chunks to start the
  stream early, 8-chunk groups at line rate, 4,2,2 so the post-compute
  drain is short).  Store descriptors are 2-8 KB runs; the 16-engine spray
  covers all 128 partitions.
"""

import sys

import numpy as np

for _p in ("/opt/trn_rl_repo", "/root/.axon_site/_ro/trn_rl_repo"):
    if _p not in sys.path:
        sys.path.append(_p)

import concourse.bass as bass
import concourse.tile as tile
from concourse import bacc, mybir
from concourse.bass_utils import run_bass_kernel_spmd

N_CORES = 8
B, NP, NFFT, SPACING = 4096, 512, 4096, 8
B_LOC = B // N_CORES  # batch rows per core
P = 128  # SBUF partitions
NCHUNK = 64  # (ch, m) chunks: ch = q // 32 (re/im), m = q % 32 (128-subcarrier block)
MG = 16  # chunk slots per band

# chunks per output store; small leading groups start the store stream as
# soon as the first psum pair is evacuated, 8-chunk middle groups give
# 8 KB-run descriptors (line rate), small trailing groups keep the
# post-compute drain short.
STORE_GROUPS = [2, 2, 4, 8, 8, 8, 8, 8, 8, 4, 2, 2]
assert sum(STORE_GROUPS) == NCHUNK

_PROGRAM = None


def _band_slot(m: int) -> tuple[int, int]:
    """SBUF band (partition base) and column slot of chunk m.

    Odd chunks (and m=31) live at partitions 0..16, even chunks at
    64..80 -- consecutive m alternate PE row-groups so back-to-back
    matmuls overlap in the array."""
    if m == 31:
        return 0, 15
    return (0, m // 2) if m % 2 else (64, m // 2)


def _build_program():
    """One Bass program, identical on all cores (pure data parallel)."""
    nc = bacc.Bacc("TRN2", target_bir_lowering=False, debug=False)
    f16 = mybir.dt.float16
    f32 = mybir.dt.float32
    # ls[34*ch + 17*band2 + j, m2*512 + b]: pre-gathered chunk layout
    # (band2 0 = odd chunks -> partitions 0..16, band2 1 = even -> 64..80).
    # Each row is one partition's contiguous 16 KB payload.
    ls = nc.dram_tensor("ls", [68, MG * B_LOC], f16, kind="ExternalInput").ap()
    # wm[81, 256]: cols 0:128 = W17 band at rows 0..16 AND 64..80 (host
    # duplicates); cols 128:256 rows 0..15 = W16 last-chunk band.
    wm = nc.dram_tensor("wm", [81, 2 * P], f16, kind="ExternalInput").ap()
    # out[p, q*512 + b]: subcarrier-position p = 8*kk + r of chunk q = ch*32 + m.
    out = nc.dram_tensor("out", [P, NCHUNK * B_LOC], f16, kind="ExternalOutput").ap()

    with tile.TileContext(nc) as tc:
        with (
            tc.tile_pool(name="wpool", bufs=1) as wpool,
            tc.tile_pool(name="lpool", bufs=1) as lpool,
            tc.psum_pool(name="ppool", bufs=4) as ppool,
            tc.tile_pool(name="opool", bufs=6) as opool,
        ):
            # Stationary weights: one DMA, 81 partitions x 512 B.  W17 sits
            # at PE base partitions 0 and 64 (cols 0:128); the hN-folded
            # last-chunk band W16 at base 0, cols 128:256.
            wb = wpool.tile([81, 2 * P], f16, name="wb", tag="wb")
            nc.sync.dma_start(wb[:], wm)

            # Data tiles: one per (ch, band).  16 slots x 512 cols fp16 =
            # 16 KB per partition.  Mains are [16, cols] (one 8/16 KB
            # descriptor per partition); the 17th row (j=16) of each band
            # goes as a [1, 16 KB] strip (flat APs spray all 16 engines).
            # 17-partition DMA destinations would break the descriptor
            # spray (HW-measured 3 engines), hence the 16+1 split.
            lts = {
                (ch, band): lpool.tile(
                    [81 if band else 17, MG * B_LOC], f16,
                    name=f"ls{ch}_{band}", tag=f"ls{ch}_{band}",
                )
                for ch in range(2)
                for band in (64, 0)
            }

            def row0(ch, band):
                return 34 * ch + (17 if band else 0)

            def main(ring, ch, band, s0, s1):
                lt = lts[(ch, band)]
                cols = slice(s0 * B_LOC, s1 * B_LOC)
                ring.dma_start(
                    lt[band : band + 16, cols],
                    ls[row0(ch, band) : row0(ch, band) + 16, cols],
                )

            def strip(ring, ch, band):
                lt = lts[(ch, band)]
                r = row0(ch, band) + 16
                ring.dma_start(lt[band + 16 : band + 17, :], ls[r : r + 1, :])

            # Load bandwidth is SBUF-AXI-PORT-limited by the destination
            # partitions: each band's 16+1 rows touch only ~5 of 16 ports
            # (~135 GB/s), and sequential pieces on ONE ring keep only one
            # band's ports busy at a time (v2: 130 GB/s aggregate, loads
            # till 17.5 us, stores FIFO-stuck behind them).  So the two
            # bands load CONCURRENTLY on different rings -- band 64 on
            # sync (HWDGE), band 0 on gpsimd (SWDGE, the ring the docs
            # measure at 341 GB/s) -- doubling the active ports.  The
            # scalar ring is NOT used: it is the slowest loader
            # (v3-measured ~90 GB/s).  ch0 mains are split s0-3/s4-7/s8-15
            # so the first chunks' data (and PE start) don't wait for a
            # full 16 KB-per-port wall; ch1 (consumed from ~20 us) loads
            # whole.  All loads drain by ~12 us, before the store stream
            # ramps, so load descriptors' non-pipelined HBM-read stalls
            # never poison store throughput on the shared SDMA engines.
            main(nc.sync, 0, 64, 0, 4)
            main(nc.gpsimd, 0, 0, 0, 4)
            strip(nc.sync, 0, 64)
            strip(nc.gpsimd, 0, 0)
            main(nc.sync, 0, 64, 4, 8)
            main(nc.gpsimd, 0, 0, 4, 8)
            main(nc.sync, 0, 64, 8, 16)
            main(nc.gpsimd, 0, 0, 8, 16)
            main(nc.sync, 1, 64, 0, 16)
            main(nc.gpsimd, 1, 0, 0, 16)
            strip(nc.sync, 1, 64)
            strip(nc.gpsimd, 1, 0)

            q = 0
            pair = 0
            for gn in STORE_GROUPS:
                o = opool.tile([P, gn * B_LOC], f16)
                q0 = q
                for j in range(0, gn, 2):
                    # one 2-bank psum tile per chunk pair: the paired evac
                    # (FD 1024) amortizes the per-op fixed cost and halves
                    # evac semaphore traffic.
                    ps = ppool.tile([P, 2 * B_LOC], f32)
                    for h in range(2):
                        ch, m = q // 32, q % 32
                        band, m2 = _band_slot(m)
                        if m == 31:
                            # last chunk: pilots 496..511, hN-folded band
                            lhsT, nrows = wb[0:16, P : 2 * P], 16
                        else:
                            lhsT, nrows = wb[band : band + 17, 0:P], 17
                        lt = lts[(ch, band)]
                        nc.tensor.matmul(
                            ps[:, h * B_LOC : (h + 1) * B_LOC],
                            lhsT,
                            lt[band : band + nrows, m2 * B_LOC : (m2 + 1) * B_LOC],
                            start=True,
                            stop=True,
                        )
                        q += 1
                    # psum -> fp16 SBUF, alternating engines so consecutive
                    # pairs drain in parallel (DVE ~1.2us, ACT ~1.1us).
                    osl = o[:, j * B_LOC : (j + 2) * B_LOC]
                    if pair % 2 == 0:
                        nc.vector.tensor_copy(osl, ps[:])
                    else:
                        nc.scalar.copy(osl, ps[:])
                    pair += 1
                nc.sync.dma_start(out[:, q0 * B_LOC : q * B_LOC], o[:])
    nc.compile()
    return nc


def _w_mats(decay_param) -> np.ndarray:
    """[81, 256] fp16 weight tensor.  Cols 0:128: W17[j, 8kk+r] = alpha[r]
    (j=kk) / gamma[r] (j=kk+1) at rows 0..16 and duplicated at rows 64..80.
    Cols 128:256 rows 0..15: the last-chunk band (kk=15 columns use the
    hN-folded coefficients on pilots 510/511)."""
    x = float(np.asarray(decay_param).reshape(-1)[0])
    d = float(np.logaddexp(0.0, x))  # softplus
    r = np.arange(SPACING, dtype=np.float64)
    eps = 1e-12
    wl = np.exp(-d * r)
    wr = np.exp(-d * (float(SPACING) - r))
    w = wl + wr + eps
    alpha, gamma = wl / w, wr / w
    # last 8 subcarriers: i = 4088 + r, x0 = 4088, x1 = 4095 (gap of 7);
    # y1 = hN = (15/8) H[511] - (7/8) H[510]
    wl2 = np.exp(-d * r)
    wr2 = np.exp(-d * (7.0 - r))
    w2 = wl2 + wr2 + eps
    c511 = (wl2 + 1.875 * wr2) / w2
    c510 = -0.875 * wr2 / w2
    W17 = np.zeros((17, P), np.float64)
    Wlast = np.zeros((16, P), np.float64)
    cols = np.arange(SPACING)
    for kk in range(16):
        W17[kk, 8 * kk + cols] = alpha
        W17[kk + 1, 8 * kk + cols] = gamma
    for kk in range(15):
        Wlast[kk, 8 * kk + cols] = alpha
        Wlast[kk + 1, 8 * kk + cols] = gamma
    Wlast[14, 120:128] = c510
    Wlast[15, 120:128] = c511
    W = np.zeros((81, 2 * P), np.float16)
    W[0:17, 0:P] = W17.astype(np.float16)
    W[64:81, 0:P] = W17.astype(np.float16)
    W[0:16, P : 2 * P] = Wlast.astype(np.float16)
    return W


def _gather_ls4(shard: np.ndarray) -> np.ndarray:
    """[68, 8192] fp16: row 34*ch + 17*band2 + j holds partition (band2 ?
    0 : 64) + j's payload -- chunk slots side by side, one contiguous 16 KB
    DRAM run per partition (big-descriptor loads)."""
    lsT = shard.transpose(2, 1, 0).astype(np.float16).reshape(2 * NP, B_LOC)
    j = np.arange(17)[:, None]  # [17, 1]
    m_odd = np.array([2 * m2 + 1 for m2 in range(15)] + [31])  # band2=0 slots
    m_even = np.arange(0, 32, 2)  # band2=1 slots
    rows_odd = np.minimum(16 * m_odd[None, :] + j, 2 * NP // 2 - 1)  # clip m31 j=16
    rows_even = 16 * m_even[None, :] + j
    out = np.empty((68, MG * B_LOC), np.float16)
    for ch in range(2):
        base = 512 * ch
        out[34 * ch : 34 * ch + 17] = lsT[base + rows_odd].reshape(17, -1)
        out[34 * ch + 17 : 34 * ch + 34] = lsT[base + rows_even].reshape(17, -1)
    return out


def kernel(LS_ri, pilot_pos=None, decay_param=None, Nfft=None, **_unused):
    global _PROGRAM
    LS_ri = np.asarray(LS_ri, dtype=np.float32)
    Wm = _w_mats(decay_param)

    if _PROGRAM is None:
        _PROGRAM = _build_program()
    nc = _PROGRAM

    in_maps = []
    for c in range(N_CORES):
        shard = LS_ri[c * B_LOC : (c + 1) * B_LOC]  # [512, 512, 2]
        in_maps.append({"ls": _gather_ls4(shard), "wm": Wm})

    res = run_bass_kernel_spmd(nc, in_maps, list(range(N_CORES))).results
    outs = []
    for c in range(N_CORES):
        a = np.asarray(res[c]["out"]).reshape(16, 8, 2, 32, B_LOC)  # kk r ch m b
        a = a.transpose(4, 3, 0, 1, 2).reshape(B_LOC, NFFT, 2)
        outs.append(a.astype(np.float32))
    return np.concatenate(outs, axis=0)
